# revision 12
# baseline (speedup 1.0000x reference)
"""3-layer GAT (N=50000, E=1.6M, Cora dims) on 8 Trainium2 NeuronCores.

Sharding: dst-node partitioned (graph parallel). Per layer:
  node phase:  h = x @ W (PE), per-node attention terms alpha_src/alpha_dst,
               pack per-node gather-table row [ (1,h_head)xH | alpha_src | pad ]
               = 128 bf16 = 256B.  AllGather table across the 8 cores.
  edge phase:  per 32-dst-node window, 128-edge tiles; dma_gather fetches
               src rows (table split in halves for int16 indices); one-hot S
               built by is_equal vs iota; PE transpose of S + block-diagonal
               matmul broadcasts alpha_dst to edges; ACT exp(leaky_relu);
               PE aggregates S^T @ (p * row) into window PSUM — the "1"
               columns of the table produce the softmax denominator Z.
  final:       normalize by Z, bias, elu, (layer3: mean heads + log_softmax).
Host does only sharding/index prep/unshard; all FLOPs on device.
"""
import sys

for _p in ("/opt/trn_rl_repo",):
    if _p not in sys.path:
        sys.path.insert(0, _p)

import numpy as np
import ml_dtypes

import concourse.bass as bass
import concourse.bacc as bacc
import concourse.tile as tile
import concourse.mybir as mybir
from concourse import bass_utils

F32 = mybir.dt.float32
BF16 = mybir.dt.bfloat16
I16 = mybir.dt.int16
AX = mybir.AxisListType
AF = mybir.ActivationFunctionType
OP = mybir.AluOpType
BF = ml_dtypes.bfloat16

NEG_SLOPE = 0.2


class Cfg:
    pass


def make_cfg(n=50000, f_in=1433, ncores=8, split=32768, sb_windows=8):
    cfg = Cfg()
    cfg.n = n
    cfg.ncores = ncores
    cfg.n_real = n // ncores
    assert cfg.n_real * ncores == n
    cfg.npc = ((cfg.n_real + 127) // 128) * 128
    cfg.ntiles = cfg.npc // 128
    cfg.win = 32
    cfg.nwin = cfg.npc // cfg.win
    cfg.f_in = f_in
    cfg.f_pad = ((f_in + 127) // 128) * 128
    cfg.kchunks = cfg.f_pad // 128
    cfg.split = split
    cfg.sb_windows = sb_windows
    cfg.nrows_g = cfg.npc * ncores
    cfg.layers = [(4, 16, f_in), (4, 16, 64), (6, 7, 64)]
    return cfg


def prep_edges(cfg, edge_index):
    n, ncores = cfg.n, cfg.ncores
    src = np.concatenate([np.asarray(edge_index[0]), np.arange(n)]).astype(np.int64)
    dst = np.concatenate([np.asarray(edge_index[1]), np.arange(n)]).astype(np.int64)
    gsrc = (src // cfg.n_real) * cfg.npc + (src % cfg.n_real)
    core_of = dst // cfg.n_real
    loc = dst % cfg.n_real
    wi = loc // cfg.win
    rel = (loc % cfg.win).astype(np.float32)

    order = np.lexsort((wi, core_of))
    gsrc_s, rel_s, wi_s, core_s = gsrc[order], rel[order], wi[order], core_of[order]
    islo_s = gsrc_s < cfg.split
    lo_e = [[None] * cfg.nwin for _ in range(ncores)]
    hi_e = [[None] * cfg.nwin for _ in range(ncores)]
    for c in range(ncores):
        cm = core_s == c
        gc, rc, wc, lc = gsrc_s[cm], rel_s[cm], wi_s[cm], islo_s[cm]
        for w in range(cfg.nwin):
            wm = wc == w
            gw, rw, lw = gc[wm], rc[wm], lc[wm]
            lo_e[c][w] = (gw[lw], rw[lw])
            hi_e[c][w] = (gw[~lw] - cfg.split, rw[~lw])

    cdiv = lambda a, b: (a + b - 1) // b
    cfg.tlo = [max(cdiv(len(lo_e[c][w][0]), 128) for c in range(ncores))
               for w in range(cfg.nwin)]
    cfg.thi = [max(cdiv(len(hi_e[c][w][0]), 128) for c in range(ncores))
               for w in range(cfg.nwin)]

    sbs = []
    col = 0
    w = 0
    while w < cfg.nwin:
        wl = list(range(w, min(w + cfg.sb_windows, cfg.nwin)))
        sb = Cfg()
        sb.windows = wl
        sb.col0 = col
        sb.lo_cols = sum(cfg.tlo[x] for x in wl)
        sb.hi_cols = sum(cfg.thi[x] for x in wl)
        sb.w_lo = {}
        sb.w_hi = {}
        c0 = col
        for x in wl:
            sb.w_lo[x] = (c0, cfg.tlo[x]); c0 += cfg.tlo[x]
        for x in wl:
            sb.w_hi[x] = (c0, cfg.thi[x]); c0 += cfg.thi[x]
        col = c0
        sbs.append(sb)
        w += cfg.sb_windows
    cfg.sbs = sbs
    cfg.ncols = col

    gidx = np.zeros((ncores, 128, cfg.ncols * 8), np.int16)
    dstrel = np.full((ncores, 128, cfg.ncols), -1.0, np.float32)
    for c in range(ncores):
        for sb in sbs:
            for half, we in ((0, sb.w_lo), (1, sb.w_hi)):
                for x, (c0, nt) in we.items():
                    if nt == 0:
                        continue
                    g, r = (lo_e[c][x] if half == 0 else hi_e[c][x])
                    cnt = len(g)
                    gpad = np.zeros(nt * 128, np.int64)
                    gpad[:cnt] = g
                    rpad = np.full(nt * 128, -1.0, np.float32)
                    rpad[:cnt] = r
                    dstrel[c, :, c0:c0 + nt] = rpad.reshape(nt, 128).T
                    wrapped = gpad.reshape(nt * 8, 16).T  # [16, nt*8]
                    gidx[c, :, c0 * 8:(c0 + nt) * 8] = np.tile(wrapped, (8, 1))
    cfg.gidx = gidx
    cfg.dstrel = dstrel
    return cfg


# ---------------------------------------------------------------------------

def build(cfg):
    nc = bacc.Bacc("TRN2", target_bir_lowering=False, debug=False,
                   num_devices=cfg.ncores)
    L3H, L3O, _ = cfg.layers[2]
    C = L3O
    HPs = [4 * ((H + 3) // 4) for (H, O, _) in cfg.layers]
    aggw = [H * (O + 1) for (H, O, _) in cfg.layers]

    xT = nc.dram_tensor("xT", [cfg.f_pad, cfg.npc], BF16, kind="ExternalInput")
    w1 = nc.dram_tensor("w1", [cfg.f_pad, 64], BF16, kind="ExternalInput")
    w2 = nc.dram_tensor("w2", [64, 64], F32, kind="ExternalInput")
    w3 = nc.dram_tensor("w3", [64, L3H * L3O], F32, kind="ExternalInput")
    abt = {}
    for l, (H, O, _) in enumerate(cfg.layers):
        abt[l] = (nc.dram_tensor(f"a{l}s", [128, H * O], F32, kind="ExternalInput"),
                  nc.dram_tensor(f"a{l}d", [128, H * O], F32, kind="ExternalInput"))
    bts = [nc.dram_tensor("b1", [128, 64], F32, kind="ExternalInput"),
           nc.dram_tensor("b2", [128, 64], F32, kind="ExternalInput"),
           nc.dram_tensor("b3", [128, C], F32, kind="ExternalInput")]
    ident_bf = nc.dram_tensor("ident_bf", [128, 128], BF16, kind="ExternalInput")
    ident_f = nc.dram_tensor("ident_f", [128, 128], F32, kind="ExternalInput")
    iota_bf = nc.dram_tensor("iota_bf", [128, cfg.win], BF16, kind="ExternalInput")
    gidx = nc.dram_tensor("gidx", [128, cfg.ncols * 8], I16, kind="ExternalInput")
    dstrel = nc.dram_tensor("dstrel", [128, cfg.ncols], BF16, kind="ExternalInput")
    out = nc.dram_tensor("out", [cfg.npc, C], F32, kind="ExternalOutput")

    SBC = max(sb.lo_cols + sb.hi_cols for sb in cfg.sbs)
    W = cfg.win

    with tile.TileContext(nc) as tc:
        with tc.tile_pool(name="dram", bufs=1, space="DRAM") as dp, \
             tc.tile_pool(name="cs", bufs=1) as cp, \
             tc.tile_pool(name="sp", bufs=3) as sp, \
             tc.tile_pool(name="gp", bufs=2) as gp, \
             tc.tile_pool(name="pp", bufs=2, space="PSUM") as pp:

            tbl_c = [dp.tile([cfg.npc, 128], BF16, tag=f"tbl{l}", name=f"tbl_c{l}")
                     for l in range(3)]
            tbl_g = [dp.tile([cfg.nrows_g, 128], BF16, addr_space="Shared",
                             tag=f"tblg{l}", name=f"tbl_g{l}") for l in range(3)]
            ad_c = [dp.tile([cfg.npc, HPs[l]], BF16, tag=f"adc{l}", name=f"ad_c{l}")
                    for l in range(3)]
            agg_c = [dp.tile([cfg.npc, aggw[l]], F32, tag=f"aggc{l}", name=f"agg_c{l}")
                     for l in range(3)]

            # ---- constants ----
            w1_sb = cp.tile([128, cfg.kchunks * 64], BF16)
            nc.sync.dma_start(
                out=w1_sb[:].rearrange("p (k o) -> p k o", o=64),
                in_=w1[:, :].rearrange("(k p) o -> p k o", p=128))
            w2_sb = cp.tile([64, 64], F32)
            nc.sync.dma_start(out=w2_sb[:], in_=w2[:, :])
            w3_sb = cp.tile([64, L3H * L3O], F32)
            nc.sync.dma_start(out=w3_sb[:], in_=w3[:, :])
            ab_sb = {}
            for l, (H, O, _) in enumerate(cfg.layers):
                s = cp.tile([128, H * O], F32, tag=f"cas{l}", name=f"as_sb{l}")
                d = cp.tile([128, H * O], F32, tag=f"cad{l}", name=f"ad_sb{l}")
                nc.sync.dma_start(out=s[:], in_=abt[l][0][:, :])
                nc.sync.dma_start(out=d[:], in_=abt[l][1][:, :])
                ab_sb[l] = (s, d)
            b_sb = []
            for l, t in enumerate(bts):
                bt = cp.tile([128, t.shape[1]], F32, tag=f"cb{l}", name=f"b_sb{l}")
                nc.sync.dma_start(out=bt[:], in_=t[:, :])
                b_sb.append(bt)
            id_bf = cp.tile([128, 128], BF16)
            nc.sync.dma_start(out=id_bf[:], in_=ident_bf[:, :])
            id_f = cp.tile([128, 128], F32)
            nc.sync.dma_start(out=id_f[:], in_=ident_f[:, :])
            iota = cp.tile([128, W], BF16)
            nc.sync.dma_start(out=iota[:], in_=iota_bf[:, :])
            gidx_sb = cp.tile([128, cfg.ncols * 8], I16)
            nc.sync.dma_start(out=gidx_sb[:], in_=gidx[:, :])
            dstrel_sb = cp.tile([128, cfg.ncols], BF16)
            nc.sync.dma_start(out=dstrel_sb[:], in_=dstrel[:, :])

            def elu_inplace(x, width):
                m = sp.tile([128, 128], F32, tag="elu_m")
                e = sp.tile([128, 128], F32, tag="elu_e")
                nc.vector.tensor_scalar_min(out=m[:, :width], in0=x, scalar1=0.0)
                nc.scalar.activation(e[:, :width], m[:, :width], AF.Exp)
                nc.vector.tensor_scalar_max(out=x, in0=x, scalar1=0.0)
                nc.vector.tensor_tensor(out=x, in0=x, in1=e[:, :width], op=OP.add)
                nc.vector.tensor_scalar_add(out=x, in0=x, scalar1=-1.0)

            def node_phase(l):
                H, O, _ = cfg.layers[l]
                OO = O + 1
                itl = H * OO
                HP = HPs[l]
                for nt in range(cfg.ntiles):
                    r0 = nt * 128
                    if l == 0:
                        h_ps = pp.tile([128, 64], F32, tag="h", bufs=1)
                        for kc in range(cfg.kchunks):
                            xt = sp.tile([128, 128], BF16, tag="xt")
                            nc.sync.dma_start(
                                out=xt[:],
                                in_=xT[kc * 128:(kc + 1) * 128, r0:r0 + 128])
                            nc.tensor.matmul(
                                out=h_ps[:], lhsT=xt[:],
                                rhs=w1_sb[:, kc * 64:(kc + 1) * 64],
                                start=(kc == 0), stop=(kc == cfg.kchunks - 1))
                    else:
                        Hp, Op, _ = cfg.layers[l - 1]
                        OOp = Op + 1
                        ag = sp.tile([128, aggw[l - 1]], F32, tag="ag")
                        nc.sync.dma_start(out=ag[:], in_=agg_c[l - 1][r0:r0 + 128, :])
                        rz = sp.tile([128, 8], F32, tag="rz")
                        nc.vector.tensor_scalar_add(
                            out=rz[:, :Hp],
                            in0=ag[:].rearrange("p (h j) -> p h j", j=OOp)[:, :, 0],
                            scalar1=1e-16)
                        nc.vector.reciprocal(out=rz[:, :Hp], in_=rz[:, :Hp])
                        x2 = sp.tile([128, Hp * Op], F32, tag="x2")
                        nc.vector.tensor_tensor(
                            out=x2[:].rearrange("p (h o) -> p h o", o=Op),
                            in0=ag[:].rearrange("p (h j) -> p h j", j=OOp)[:, :, 1:OOp],
                            in1=rz[:, :Hp].unsqueeze(2).broadcast_to([128, Hp, Op]),
                            op=OP.mult)
                        nc.vector.tensor_tensor(out=x2[:], in0=x2[:],
                                                in1=b_sb[l - 1][:], op=OP.add)
                        elu_inplace(x2[:], Hp * Op)
                        x2t_ps = pp.tile([64, 128], F32, tag="x2t", bufs=1)
                        nc.tensor.transpose(out=x2t_ps[:], in_=x2[:], identity=id_f[:])
                        x2t = sp.tile([64, 128], F32, tag="x2ts")
                        nc.vector.tensor_copy(out=x2t[:], in_=x2t_ps[:])
                        h_ps = pp.tile([128, 64], F32, tag="h", bufs=1)
                        nc.tensor.matmul(out=h_ps[:, :H * O], lhsT=x2t[:],
                                         rhs=(w2_sb if l == 1 else w3_sb)[:],
                                         start=True, stop=True)
                    h_ap = h_ps[:, :H * O]
                    # table row assembly
                    tbl = sp.tile([128, 128], BF16, tag="tbl")
                    nc.gpsimd.memset(tbl[:, itl + H:128], 0.0)
                    nc.gpsimd.memset(
                        tbl[:, :itl].rearrange("p (h j) -> p h j", j=OO)[:, :, 0:1],
                        1.0)
                    nc.vector.tensor_copy(
                        out=tbl[:, :itl].rearrange("p (h j) -> p h j", j=OO)[:, :, 1:OO],
                        in_=h_ap.rearrange("p (h o) -> p h o", o=O))
                    ts = sp.tile([128, 64], F32, tag="ts")
                    asf = sp.tile([128, 8], F32, tag="asf")
                    nc.vector.tensor_tensor(out=ts[:, :H * O], in0=h_ap,
                                            in1=ab_sb[l][0][:], op=OP.mult)
                    nc.vector.reduce_sum(
                        asf[:, :H],
                        ts[:, :H * O].rearrange("p (h o) -> p h o", o=O), AX.X)
                    nc.vector.tensor_copy(out=tbl[:, itl:itl + H], in_=asf[:, :H])
                    nc.sync.dma_start(out=tbl_c[l][r0:r0 + 128, :], in_=tbl[:])
                    # alpha_dst
                    nc.vector.tensor_tensor(out=ts[:, :H * O], in0=h_ap,
                                            in1=ab_sb[l][1][:], op=OP.mult)
                    nc.vector.reduce_sum(
                        asf[:, :H],
                        ts[:, :H * O].rearrange("p (h o) -> p h o", o=O), AX.X)
                    adp = sp.tile([128, 8], BF16, tag="adp")
                    if HP > H:
                        nc.gpsimd.memset(adp[:, :HP], 0.0)
                    nc.vector.tensor_copy(out=adp[:, :H], in_=asf[:, :H])
                    nc.sync.dma_start(out=ad_c[l][r0:r0 + 128, :], in_=adp[:, :HP])

            def edge_phase(l):
                parts = getattr(cfg, "edge_parts",
                                {"S", "T", "ad", "agg"})
                H, O, _ = cfg.layers[l]
                OO = O + 1
                itl = H * OO
                HP = HPs[l]
                for sb in cfg.sbs:
                    ncols_sb = sb.lo_cols + sb.hi_cols
                    nwsb = len(sb.windows)
                    gbuf = gp.tile([128, SBC, 128], BF16, tag="gbuf")
                    if sb.lo_cols:
                        nc.gpsimd.dma_gather(
                            gbuf[:, 0:sb.lo_cols, :],
                            tbl_g[l][0:min(cfg.split, cfg.nrows_g), :],
                            gidx_sb[:, sb.col0 * 8:(sb.col0 + sb.lo_cols) * 8],
                            num_idxs=sb.lo_cols * 128,
                            num_idxs_reg=sb.lo_cols * 128,
                            elem_size=128, single_packet=False)
                    if sb.hi_cols:
                        nc.gpsimd.dma_gather(
                            gbuf[:, sb.lo_cols:ncols_sb, :],
                            tbl_g[l][cfg.split:cfg.nrows_g, :],
                            gidx_sb[:, (sb.col0 + sb.lo_cols) * 8:
                                    (sb.col0 + ncols_sb) * 8],
                            num_idxs=sb.hi_cols * 128,
                            num_idxs_reg=sb.hi_cols * 128,
                            elem_size=128, single_packet=False)
                    # alpha_dst, replicated into the 4 partition blocks
                    ad4 = sp.tile([128, cfg.sb_windows * 8], BF16, tag="ad4")
                    for t4 in range(4):
                        nc.sync.dma_start(
                            out=ad4[t4 * W:(t4 + 1) * W, 0:nwsb * HP].rearrange(
                                "r (w h) -> r w h", h=HP),
                            in_=ad_c[l][:, :].rearrange(
                                "(w r) h -> r w h", r=W)[
                                :, sb.windows[0]:sb.windows[0] + nwsb, :])
                    for x in sb.windows:
                        runs = []
                        c0, nlo = sb.w_lo[x]
                        if nlo:
                            runs.append((c0, nlo))
                        c0, nhi = sb.w_hi[x]
                        if nhi:
                            runs.append((c0, nhi))
                        ntile_w = sum(r[1] for r in runs)
                        if ntile_w == 0:
                            aw = sp.tile([W, 128], F32, tag="aw")
                            nc.gpsimd.memset(aw[:, :itl], 0.0)
                            nc.sync.dma_start(
                                out=agg_c[l][x * W:(x + 1) * W, :],
                                in_=aw[:, :itl])
                            continue
                        if not parts:
                            aw = sp.tile([W, 128], F32, tag="aw")
                            nc.gpsimd.memset(aw[:, :itl], 0.0)
                            nc.sync.dma_start(
                                out=agg_c[l][x * W:(x + 1) * W, :],
                                in_=aw[:, :itl])
                            continue
                        psum = pp.tile([W, itl], F32, tag="agg")
                        wl = x - sb.windows[0]
                        bd = sp.tile([128, 4 * 8], BF16, tag="bd")
                        nc.gpsimd.memset(bd[:], 0.0)
                        for t4 in range(4):
                            nc.vector.tensor_copy(
                                out=bd[t4 * W:(t4 + 1) * W,
                                       t4 * HP:(t4 + 1) * HP],
                                in_=ad4[t4 * W:(t4 + 1) * W,
                                        wl * HP:(wl + 1) * HP])
                        ti = 0
                        first = True
                        for (rc0, rn) in runs:
                            for g0 in range(0, rn, 4):
                                g = min(4, rn - g0)
                                cols = rc0 + g0 - sb.col0
                                S = sp.tile([128, 4 * W], BF16, tag="S")
                                nc.vector.tensor_tensor(
                                    out=S[:, 0:g * W].rearrange(
                                        "p (t w) -> p t w", w=W),
                                    in0=dstrel_sb[:, rc0 + g0:rc0 + g0 + g]
                                    .unsqueeze(2).broadcast_to([128, g, W]),
                                    in1=iota[:].unsqueeze(1)
                                    .broadcast_to([128, g, W]),
                                    op=OP.is_equal)
                                if "T" not in parts:
                                    continue
                                st_ps = pp.tile([128, 128], BF16, tag="st")
                                nc.tensor.transpose(out=st_ps[0:g * W, :],
                                                    in_=S[:, 0:g * W],
                                                    identity=id_bf[:])
                                st = sp.tile([128, 128], BF16, tag="sts")
                                nc.vector.tensor_copy(out=st[0:g * W, :],
                                                      in_=st_ps[0:g * W, :])
                                if "ad" not in parts:
                                    continue
                                ade_ps = pp.tile([128, 4 * 8], F32, tag="ade")
                                nc.tensor.matmul(out=ade_ps[:, 0:g * HP],
                                                 lhsT=st[0:g * W, :],
                                                 rhs=bd[0:g * W, 0:g * HP],
                                                 start=True, stop=True)
                                u = sp.tile([128, 4 * 8], F32, tag="u")
                                nc.vector.tensor_tensor(
                                    out=u[:, 0:g * H].rearrange(
                                        "p (t h) -> p t h", h=H),
                                    in0=gbuf[:, cols:cols + g, itl:itl + H],
                                    in1=ade_ps[:, 0:g * HP].rearrange(
                                        "p (t h) -> p t h", h=HP)[:, :, 0:H],
                                    op=OP.add)
                                nc.vector.scalar_tensor_tensor(
                                    out=u[:, 0:g * H], in0=u[:, 0:g * H],
                                    scalar=NEG_SLOPE, in1=u[:, 0:g * H],
                                    op0=OP.mult, op1=OP.max)
                                p = sp.tile([128, 4 * 8], BF16, tag="p")
                                nc.scalar.activation(p[:, 0:g * H], u[:, 0:g * H],
                                                     AF.Exp)
                                msg = sp.tile([128, 4 * 85], BF16, tag="msg")
                                nc.vector.tensor_tensor(
                                    out=msg[:, 0:g * itl].rearrange(
                                        "p (t h j) -> p t h j", h=H, j=OO),
                                    in0=gbuf[:, cols:cols + g, 0:itl].rearrange(
                                        "p t (h j) -> p t h j", j=OO),
                                    in1=p[:, 0:g * H].rearrange(
                                        "p (t h) -> p t h", h=H)
                                    .unsqueeze(3).broadcast_to([128, g, H, OO]),
                                    op=OP.mult)
                                if "agg" not in parts:
                                    continue
                                for t in range(g):
                                    nc.tensor.matmul(
                                        out=psum[:, :],
                                        lhsT=S[:, t * W:(t + 1) * W],
                                        rhs=msg[:, t * itl:(t + 1) * itl],
                                        start=first,
                                        stop=(ti == ntile_w - 1))
                                    first = False
                                    ti += 1
                        aw = sp.tile([W, 128], F32, tag="aw")
                        if "agg" in parts:
                            nc.vector.tensor_copy(out=aw[:, :itl], in_=psum[:, :])
                        else:
                            nc.gpsimd.memset(aw[:, :itl], 0.0)
                        nc.sync.dma_start(
                            out=agg_c[l][x * W:(x + 1) * W, :],
                            in_=aw[:, :itl])

            def final_phase():
                H, O, _ = cfg.layers[2]
                OO = O + 1
                for nt in range(cfg.ntiles):
                    r0 = nt * 128
                    ag = sp.tile([128, aggw[2]], F32, tag="ag")
                    nc.sync.dma_start(out=ag[:], in_=agg_c[2][r0:r0 + 128, :])
                    rz = sp.tile([128, 8], F32, tag="rz")
                    nc.vector.tensor_scalar_add(
                        out=rz[:, :H],
                        in0=ag[:].rearrange("p (h j) -> p h j", j=OO)[:, :, 0],
                        scalar1=1e-16)
                    nc.vector.reciprocal(out=rz[:, :H], in_=rz[:, :H])
                    fm = sp.tile([128, O * H], F32, tag="fm")
                    nc.vector.tensor_tensor(
                        out=fm[:].rearrange("p (c h) -> p c h", h=H),
                        in0=ag[:].rearrange("p (h j) -> p h j", j=OO)[:, :, 1:OO]
                        .rearrange("p h c -> p c h"),
                        in1=rz[:, :H].unsqueeze(1).broadcast_to([128, O, H]),
                        op=OP.mult)
                    y = sp.tile([128, O], F32, tag="y")
                    nc.vector.reduce_sum(
                        y[:], fm[:].rearrange("p (c h) -> p c h", h=H), AX.X)
                    nc.vector.tensor_scalar_mul(out=y[:], in0=y[:], scalar1=1.0 / H)
                    nc.vector.tensor_tensor(out=y[:], in0=y[:], in1=b_sb[2][:],
                                            op=OP.add)
                    elu_inplace(y[:], O)
                    mx = sp.tile([128, 1], F32, tag="mx")
                    nc.vector.reduce_sum(mx[:], y[:], AX.X, op=OP.max)
                    nmx = sp.tile([128, 1], F32, tag="nmx")
                    nc.vector.tensor_scalar_mul(out=nmx[:], in0=mx[:], scalar1=-1.0)
                    ey = sp.tile([128, O], F32, tag="ey")
                    nc.scalar.activation(ey[:], y[:], AF.Exp, bias=nmx[:])
                    se = sp.tile([128, 1], F32, tag="se")
                    nc.vector.reduce_sum(se[:], ey[:], AX.X)
                    ls = sp.tile([128, 1], F32, tag="ls")
                    nc.scalar.activation(ls[:], se[:], AF.Ln)
                    nc.vector.tensor_tensor(out=ls[:], in0=ls[:], in1=mx[:],
                                            op=OP.add)
                    yo = sp.tile([128, O], F32, tag="yo")
                    nc.vector.tensor_tensor(
                        out=yo[:], in0=y[:],
                        in1=ls[:].broadcast_to([128, O]), op=OP.subtract)
                    nc.sync.dma_start(out=out[r0:r0 + 128, :], in_=yo[:])

            stop = getattr(cfg, "stop_after", "")
            done = False
            for l in range(3):
                node_phase(l)
                if stop == f"node{l}":
                    done = True; break
                nc.gpsimd.collective_compute(
                    "AllGather", OP.bypass,
                    replica_groups=[list(range(cfg.ncores))],
                    ins=[tbl_c[l][:, :]], outs=[tbl_g[l][:, :]])
                if stop == f"ag{l}":
                    done = True; break
                edge_phase(l)
                if stop == f"edge{l}":
                    done = True; break
            if not done:
                final_phase()

    nc.compile()
    return nc


# ---------------------------------------------------------------------------

def _pad_rows(a, rows):
    o = np.zeros((rows,) + a.shape[1:], np.float32)
    o[:a.shape[0]] = a
    return o


def make_in_maps(cfg, x, Ws, As, Bs):
    W1, W2, W3 = Ws
    ident = np.eye(128, dtype=np.float32)
    iota = np.tile(np.arange(cfg.win, dtype=np.float32), (128, 1))
    xT = np.ascontiguousarray(np.asarray(x, np.float32).T)
    in_maps = []
    for c in range(cfg.ncores):
        m = {}
        xp = np.zeros((cfg.f_pad, cfg.npc), np.float32)
        xp[:cfg.f_in, :cfg.n_real] = xT[:, c * cfg.n_real:(c + 1) * cfg.n_real]
        m["xT"] = xp.astype(BF)
        m["w1"] = _pad_rows(W1, cfg.f_pad).astype(BF)
        m["w2"] = np.asarray(W2, np.float32)
        m["w3"] = np.asarray(W3, np.float32)
        for l, (asv, adv) in enumerate(As):
            m[f"a{l}s"] = np.tile(asv.reshape(1, -1), (128, 1)).astype(np.float32)
            m[f"a{l}d"] = np.tile(adv.reshape(1, -1), (128, 1)).astype(np.float32)
        for i, b in enumerate(Bs):
            m[f"b{i + 1}"] = np.tile(np.asarray(b, np.float32).reshape(1, -1),
                                     (128, 1))
        m["ident_bf"] = ident.astype(BF)
        m["ident_f"] = ident
        m["iota_bf"] = iota.astype(BF)
        m["gidx"] = cfg.gidx[c]
        m["dstrel"] = cfg.dstrel[c].astype(BF)
        in_maps.append(m)
    return in_maps


_CACHE = {}


def run_gat(cfg, nc, x, Ws, As, Bs, **kw):
    in_maps = make_in_maps(cfg, x, Ws, As, Bs)
    res = bass_utils.run_bass_kernel_spmd(nc, in_maps,
                                          core_ids=list(range(cfg.ncores)), **kw)
    outs = [np.asarray(res.results[c]["out"][:cfg.n_real], np.float32)
            for c in range(cfg.ncores)]
    return np.concatenate(outs, axis=0), res


def kernel(x, edge_index, W1, a1s, a1d, b1, W2, a2s, a2d, b2, W3, a3s, a3d, b3):
    x = np.asarray(x, np.float32)
    if "full" not in _CACHE:
        cfg = make_cfg(n=x.shape[0], f_in=x.shape[1])
        prep_edges(cfg, np.asarray(edge_index))
        nc = build(cfg)
        _CACHE["full"] = (cfg, nc)
    cfg, nc = _CACHE["full"]
    As = [(np.asarray(a, np.float32).reshape(-1), np.asarray(d, np.float32).reshape(-1))
          for a, d in ((a1s, a1d), (a2s, a2d), (a3s, a3d))]
    out, _ = run_gat(cfg, nc, x, (np.asarray(W1, np.float32),
                                  np.asarray(W2, np.float32),
                                  np.asarray(W3, np.float32)),
                     As, (b1, b2, b3))
    return out


# revision 13
# speedup vs baseline: 8.8104x; 8.8104x over previous
"""3-layer GAT (N=50000, E=1.6M, Cora dims) on 8 Trainium2 NeuronCores.

Sharding: dst-node partitioned (graph parallel). Per layer:
  node phase:  h = x @ W (PE), per-node attention terms alpha_src/alpha_dst,
               pack per-node gather-table row [ (1,h_head)xH | alpha_src | pad ]
               = 128 bf16 = 256B.  AllGather table across the 8 cores.
  edge phase:  per 32-dst-node window, 128-edge tiles; dma_gather fetches
               src rows (table split in halves for int16 indices); one-hot S
               built by is_equal vs iota; PE transpose of S + block-diagonal
               matmul broadcasts alpha_dst to edges; ACT exp(leaky_relu);
               PE aggregates S^T @ (p * row) into window PSUM — the "1"
               columns of the table produce the softmax denominator Z.
  final:       normalize by Z, bias, elu, (layer3: mean heads + log_softmax).
Host does only sharding/index prep/unshard; all FLOPs on device.
"""
import sys

for _p in ("/opt/trn_rl_repo",):
    if _p not in sys.path:
        sys.path.insert(0, _p)

import numpy as np
import ml_dtypes

import concourse.bass as bass
import concourse.bacc as bacc
import concourse.tile as tile
import concourse.mybir as mybir
from concourse import bass_utils

F32 = mybir.dt.float32
BF16 = mybir.dt.bfloat16
I16 = mybir.dt.int16
AX = mybir.AxisListType
AF = mybir.ActivationFunctionType
OP = mybir.AluOpType
BF = ml_dtypes.bfloat16

NEG_SLOPE = 0.2


class Cfg:
    pass


def make_cfg(n=50000, f_in=1433, ncores=8, split=32768, sb_windows=8):
    cfg = Cfg()
    cfg.n = n
    cfg.ncores = ncores
    cfg.n_real = n // ncores
    assert cfg.n_real * ncores == n
    cfg.npc = ((cfg.n_real + 127) // 128) * 128
    cfg.ntiles = cfg.npc // 128
    cfg.win = 32
    cfg.nwin = cfg.npc // cfg.win
    cfg.f_in = f_in
    cfg.f_pad = ((f_in + 127) // 128) * 128
    cfg.kchunks = cfg.f_pad // 128
    cfg.split = split
    cfg.sb_windows = sb_windows
    cfg.nrows_g = cfg.npc * ncores
    cfg.layers = [(4, 16, f_in), (4, 16, 64), (6, 7, 64)]
    return cfg


def prep_edges(cfg, edge_index):
    n, ncores = cfg.n, cfg.ncores
    src = np.concatenate([np.asarray(edge_index[0]), np.arange(n)]).astype(np.int64)
    dst = np.concatenate([np.asarray(edge_index[1]), np.arange(n)]).astype(np.int64)
    gsrc = (src // cfg.n_real) * cfg.npc + (src % cfg.n_real)
    core_of = dst // cfg.n_real
    loc = dst % cfg.n_real
    wi = loc // cfg.win
    rel = (loc % cfg.win).astype(np.float32)

    order = np.lexsort((wi, core_of))
    gsrc_s, rel_s, wi_s, core_s = gsrc[order], rel[order], wi[order], core_of[order]
    islo_s = gsrc_s < cfg.split
    lo_e = [[None] * cfg.nwin for _ in range(ncores)]
    hi_e = [[None] * cfg.nwin for _ in range(ncores)]
    for c in range(ncores):
        cm = core_s == c
        gc, rc, wc, lc = gsrc_s[cm], rel_s[cm], wi_s[cm], islo_s[cm]
        for w in range(cfg.nwin):
            wm = wc == w
            gw, rw, lw = gc[wm], rc[wm], lc[wm]
            lo_e[c][w] = (gw[lw], rw[lw])
            hi_e[c][w] = (gw[~lw] - cfg.split, rw[~lw])

    cdiv = lambda a, b: (a + b - 1) // b
    cfg.tlo = [max(cdiv(len(lo_e[c][w][0]), 128) for c in range(ncores))
               for w in range(cfg.nwin)]
    cfg.thi = [max(cdiv(len(hi_e[c][w][0]), 128) for c in range(ncores))
               for w in range(cfg.nwin)]

    sbs = []
    col = 0
    w = 0
    while w < cfg.nwin:
        wl = list(range(w, min(w + cfg.sb_windows, cfg.nwin)))
        sb = Cfg()
        sb.windows = wl
        sb.col0 = col
        sb.lo_cols = sum(cfg.tlo[x] for x in wl)
        sb.hi_cols = sum(cfg.thi[x] for x in wl)
        sb.w_lo = {}
        sb.w_hi = {}
        c0 = col
        for x in wl:
            sb.w_lo[x] = (c0, cfg.tlo[x]); c0 += cfg.tlo[x]
        for x in wl:
            sb.w_hi[x] = (c0, cfg.thi[x]); c0 += cfg.thi[x]
        col = c0
        sbs.append(sb)
        w += cfg.sb_windows
    cfg.sbs = sbs
    cfg.ncols = col

    gidx = np.zeros((ncores, 128, cfg.ncols * 8), np.int16)
    dstrel = np.full((ncores, 128, cfg.ncols), -1.0, np.float32)
    for c in range(ncores):
        for sb in sbs:
            for half, we in ((0, sb.w_lo), (1, sb.w_hi)):
                for x, (c0, nt) in we.items():
                    if nt == 0:
                        continue
                    g, r = (lo_e[c][x] if half == 0 else hi_e[c][x])
                    cnt = len(g)
                    gpad = np.zeros(nt * 128, np.int64)
                    gpad[:cnt] = g
                    rpad = np.full(nt * 128, -1.0, np.float32)
                    rpad[:cnt] = r
                    dstrel[c, :, c0:c0 + nt] = rpad.reshape(nt, 128).T
                    wrapped = gpad.reshape(nt * 8, 16).T  # [16, nt*8]
                    gidx[c, :, c0 * 8:(c0 + nt) * 8] = np.tile(wrapped, (8, 1))
    cfg.gidx = gidx
    cfg.dstrel = dstrel
    return cfg


# ---------------------------------------------------------------------------

def build(cfg):
    nc = bacc.Bacc("TRN2", target_bir_lowering=False, debug=False,
                   num_devices=cfg.ncores)
    L3H, L3O, _ = cfg.layers[2]
    C = L3O
    HPs = [4 * ((H + 3) // 4) for (H, O, _) in cfg.layers]
    aggw = [H * (O + 1) for (H, O, _) in cfg.layers]

    xT = nc.dram_tensor("xT", [cfg.f_pad, cfg.npc], BF16, kind="ExternalInput")
    w1 = nc.dram_tensor("w1", [cfg.f_pad, 64], BF16, kind="ExternalInput")
    w2 = nc.dram_tensor("w2", [64, 64], F32, kind="ExternalInput")
    w3 = nc.dram_tensor("w3", [64, L3H * L3O], F32, kind="ExternalInput")
    abt = {}
    for l, (H, O, _) in enumerate(cfg.layers):
        abt[l] = (nc.dram_tensor(f"a{l}s", [128, H * O], F32, kind="ExternalInput"),
                  nc.dram_tensor(f"a{l}d", [128, H * O], F32, kind="ExternalInput"))
    bts = [nc.dram_tensor("b1", [128, 64], F32, kind="ExternalInput"),
           nc.dram_tensor("b2", [128, 64], F32, kind="ExternalInput"),
           nc.dram_tensor("b3", [128, C], F32, kind="ExternalInput")]
    ident_bf = nc.dram_tensor("ident_bf", [128, 128], BF16, kind="ExternalInput")
    ident_f = nc.dram_tensor("ident_f", [128, 128], F32, kind="ExternalInput")
    iota_bf = nc.dram_tensor("iota_bf", [128, cfg.win], BF16, kind="ExternalInput")
    gidx = nc.dram_tensor("gidx", [128, cfg.ncols * 8], I16, kind="ExternalInput")
    dstrel = nc.dram_tensor("dstrel", [128, cfg.ncols], BF16, kind="ExternalInput")
    out = nc.dram_tensor("out", [cfg.npc, C], F32, kind="ExternalOutput")

    SBC = max(sb.lo_cols + sb.hi_cols for sb in cfg.sbs)
    W = cfg.win

    with tile.TileContext(nc) as tc:
        with tc.tile_pool(name="dram", bufs=1, space="DRAM") as dp, \
             tc.tile_pool(name="cs", bufs=1) as cp, \
             tc.tile_pool(name="sp", bufs=3) as sp, \
             tc.tile_pool(name="gp", bufs=2) as gp, \
             tc.tile_pool(name="pp", bufs=2, space="PSUM") as pp:

            tbl_c = [dp.tile([cfg.npc, 128], BF16, tag=f"tbl{l}", name=f"tbl_c{l}")
                     for l in range(3)]
            tbl_g = [dp.tile([cfg.nrows_g, 128], BF16, addr_space="Shared",
                             tag=f"tblg{l}", name=f"tbl_g{l}") for l in range(3)]
            ad_c = [dp.tile([cfg.npc, HPs[l]], BF16, tag=f"adc{l}", name=f"ad_c{l}")
                    for l in range(3)]
            agg_c = [dp.tile([cfg.npc, aggw[l]], F32, tag=f"aggc{l}", name=f"agg_c{l}")
                     for l in range(3)]

            # ---- constants ----
            w1_sb = cp.tile([128, cfg.kchunks * 64], BF16)
            nc.sync.dma_start(
                out=w1_sb[:].rearrange("p (k o) -> p k o", o=64),
                in_=w1[:, :].rearrange("(k p) o -> p k o", p=128))
            w2_sb = cp.tile([64, 64], F32)
            nc.sync.dma_start(out=w2_sb[:], in_=w2[:, :])
            w3_sb = cp.tile([64, L3H * L3O], F32)
            nc.sync.dma_start(out=w3_sb[:], in_=w3[:, :])
            ab_sb = {}
            for l, (H, O, _) in enumerate(cfg.layers):
                s = cp.tile([128, H * O], F32, tag=f"cas{l}", name=f"as_sb{l}")
                d = cp.tile([128, H * O], F32, tag=f"cad{l}", name=f"ad_sb{l}")
                nc.sync.dma_start(out=s[:], in_=abt[l][0][:, :])
                nc.sync.dma_start(out=d[:], in_=abt[l][1][:, :])
                ab_sb[l] = (s, d)
            b_sb = []
            for l, t in enumerate(bts):
                bt = cp.tile([128, t.shape[1]], F32, tag=f"cb{l}", name=f"b_sb{l}")
                nc.sync.dma_start(out=bt[:], in_=t[:, :])
                b_sb.append(bt)
            id_bf = cp.tile([128, 128], BF16)
            nc.sync.dma_start(out=id_bf[:], in_=ident_bf[:, :])
            id_f = cp.tile([128, 128], F32)
            nc.sync.dma_start(out=id_f[:], in_=ident_f[:, :])
            iota = cp.tile([128, W], BF16)
            nc.sync.dma_start(out=iota[:], in_=iota_bf[:, :])
            gidx_sb = cp.tile([128, cfg.ncols * 8], I16)
            nc.sync.dma_start(out=gidx_sb[:], in_=gidx[:, :])
            dstrel_sb = cp.tile([128, cfg.ncols], BF16)
            nc.sync.dma_start(out=dstrel_sb[:], in_=dstrel[:, :])
            bd_tiles = {}
            for l in range(3):
                bd_tiles[l] = [cp.tile([128, 4 * 8], BF16, tag=f"bd{l}_{i}",
                                       name=f"bd{l}_{i}") for i in range(4)]
                for t in bd_tiles[l]:
                    nc.vector.memset(t[:], 0.0)

            def elu_inplace(x, width):
                m = sp.tile([128, 128], F32, tag="elu_m")
                e = sp.tile([128, 128], F32, tag="elu_e")
                nc.vector.tensor_scalar_min(out=m[:, :width], in0=x, scalar1=0.0)
                nc.scalar.activation(e[:, :width], m[:, :width], AF.Exp)
                nc.vector.tensor_scalar_max(out=x, in0=x, scalar1=0.0)
                nc.vector.tensor_tensor(out=x, in0=x, in1=e[:, :width], op=OP.add)
                nc.vector.tensor_scalar_add(out=x, in0=x, scalar1=-1.0)

            def node_phase(l):
                H, O, _ = cfg.layers[l]
                OO = O + 1
                itl = H * OO
                HP = HPs[l]
                for nt in range(cfg.ntiles):
                    r0 = nt * 128
                    if l == 0:
                        h_ps = pp.tile([128, 64], F32, tag="h", bufs=1)
                        for kc in range(cfg.kchunks):
                            xt = sp.tile([128, 128], BF16, tag="xt")
                            nc.sync.dma_start(
                                out=xt[:],
                                in_=xT[kc * 128:(kc + 1) * 128, r0:r0 + 128])
                            nc.tensor.matmul(
                                out=h_ps[:], lhsT=xt[:],
                                rhs=w1_sb[:, kc * 64:(kc + 1) * 64],
                                start=(kc == 0), stop=(kc == cfg.kchunks - 1))
                    else:
                        Hp, Op, _ = cfg.layers[l - 1]
                        OOp = Op + 1
                        ag = sp.tile([128, aggw[l - 1]], F32, tag="ag")
                        nc.sync.dma_start(out=ag[:], in_=agg_c[l - 1][r0:r0 + 128, :])
                        rz = sp.tile([128, 8], F32, tag="rz")
                        nc.vector.tensor_scalar_add(
                            out=rz[:, :Hp],
                            in0=ag[:].rearrange("p (h j) -> p h j", j=OOp)[:, :, 0],
                            scalar1=1e-16)
                        nc.vector.reciprocal(out=rz[:, :Hp], in_=rz[:, :Hp])
                        x2 = sp.tile([128, Hp * Op], F32, tag="x2")
                        nc.vector.tensor_tensor(
                            out=x2[:].rearrange("p (h o) -> p h o", o=Op),
                            in0=ag[:].rearrange("p (h j) -> p h j", j=OOp)[:, :, 1:OOp],
                            in1=rz[:, :Hp].unsqueeze(2).broadcast_to([128, Hp, Op]),
                            op=OP.mult)
                        nc.vector.tensor_tensor(out=x2[:], in0=x2[:],
                                                in1=b_sb[l - 1][:], op=OP.add)
                        elu_inplace(x2[:], Hp * Op)
                        x2t_ps = pp.tile([64, 128], F32, tag="x2t", bufs=1)
                        nc.tensor.transpose(out=x2t_ps[:], in_=x2[:], identity=id_f[:])
                        x2t = sp.tile([64, 128], F32, tag="x2ts")
                        nc.vector.tensor_copy(out=x2t[:], in_=x2t_ps[:])
                        h_ps = pp.tile([128, 64], F32, tag="h", bufs=1)
                        nc.tensor.matmul(out=h_ps[:, :H * O], lhsT=x2t[:],
                                         rhs=(w2_sb if l == 1 else w3_sb)[:],
                                         start=True, stop=True)
                    h_ap = h_ps[:, :H * O]
                    # table row assembly
                    tbl = sp.tile([128, 128], BF16, tag="tbl")
                    nc.vector.memset(tbl[:, itl + H:128], 0.0)
                    nc.vector.memset(
                        tbl[:, :itl].rearrange("p (h j) -> p h j", j=OO)[:, :, 0:1],
                        1.0)
                    nc.vector.tensor_copy(
                        out=tbl[:, :itl].rearrange("p (h j) -> p h j", j=OO)[:, :, 1:OO],
                        in_=h_ap.rearrange("p (h o) -> p h o", o=O))
                    ts = sp.tile([128, 64], F32, tag="ts")
                    asf = sp.tile([128, 8], F32, tag="asf")
                    nc.vector.tensor_tensor(out=ts[:, :H * O], in0=h_ap,
                                            in1=ab_sb[l][0][:], op=OP.mult)
                    nc.vector.reduce_sum(
                        asf[:, :H],
                        ts[:, :H * O].rearrange("p (h o) -> p h o", o=O), AX.X)
                    nc.vector.tensor_copy(out=tbl[:, itl:itl + H], in_=asf[:, :H])
                    nc.sync.dma_start(out=tbl_c[l][r0:r0 + 128, :], in_=tbl[:])
                    # alpha_dst
                    nc.vector.tensor_tensor(out=ts[:, :H * O], in0=h_ap,
                                            in1=ab_sb[l][1][:], op=OP.mult)
                    nc.vector.reduce_sum(
                        asf[:, :H],
                        ts[:, :H * O].rearrange("p (h o) -> p h o", o=O), AX.X)
                    adp = sp.tile([128, 8], BF16, tag="adp")
                    if HP > H:
                        nc.vector.memset(adp[:, :HP], 0.0)
                    nc.vector.tensor_copy(out=adp[:, :H], in_=asf[:, :H])
                    nc.sync.dma_start(out=ad_c[l][r0:r0 + 128, :], in_=adp[:, :HP])

            def edge_phase(l):
                parts = getattr(cfg, "edge_parts",
                                {"S", "T", "ad", "agg"})
                H, O, _ = cfg.layers[l]
                OO = O + 1
                itl = H * OO
                HP = HPs[l]
                for sb in cfg.sbs:
                    ncols_sb = sb.lo_cols + sb.hi_cols
                    nwsb = len(sb.windows)
                    gbuf = gp.tile([128, SBC, 128], BF16, tag="gbuf")
                    if sb.lo_cols:
                        nc.gpsimd.dma_gather(
                            gbuf[:, 0:sb.lo_cols, :],
                            tbl_g[l][0:min(cfg.split, cfg.nrows_g), :],
                            gidx_sb[:, sb.col0 * 8:(sb.col0 + sb.lo_cols) * 8],
                            num_idxs=sb.lo_cols * 128,
                            num_idxs_reg=sb.lo_cols * 128,
                            elem_size=128, single_packet=False)
                    if sb.hi_cols:
                        nc.gpsimd.dma_gather(
                            gbuf[:, sb.lo_cols:ncols_sb, :],
                            tbl_g[l][cfg.split:cfg.nrows_g, :],
                            gidx_sb[:, (sb.col0 + sb.lo_cols) * 8:
                                    (sb.col0 + ncols_sb) * 8],
                            num_idxs=sb.hi_cols * 128,
                            num_idxs_reg=sb.hi_cols * 128,
                            elem_size=128, single_packet=False)
                    # alpha_dst, replicated into the 4 partition blocks
                    ad4 = sp.tile([128, cfg.sb_windows * 8], BF16, tag="ad4")
                    for t4 in range(4):
                        nc.sync.dma_start(
                            out=ad4[t4 * W:(t4 + 1) * W, 0:nwsb * HP].rearrange(
                                "r (w h) -> r w h", h=HP),
                            in_=ad_c[l][:, :].rearrange(
                                "(w r) h -> r w h", r=W)[
                                :, sb.windows[0]:sb.windows[0] + nwsb, :])
                    for x in sb.windows:
                        bd = bd_tiles[l][x % len(bd_tiles[l])]
                        runs = []
                        c0, nlo = sb.w_lo[x]
                        if nlo:
                            runs.append((c0, nlo))
                        c0, nhi = sb.w_hi[x]
                        if nhi:
                            runs.append((c0, nhi))
                        ntile_w = sum(r[1] for r in runs)
                        if ntile_w == 0:
                            aw = sp.tile([W, 128], F32, tag="aw")
                            nc.vector.memset(aw[:, :itl], 0.0)
                            nc.sync.dma_start(
                                out=agg_c[l][x * W:(x + 1) * W, :],
                                in_=aw[:, :itl])
                            continue
                        if not parts:
                            aw = sp.tile([W, 128], F32, tag="aw")
                            nc.vector.memset(aw[:, :itl], 0.0)
                            nc.sync.dma_start(
                                out=agg_c[l][x * W:(x + 1) * W, :],
                                in_=aw[:, :itl])
                            continue
                        psum = pp.tile([W, itl], F32, tag="agg")
                        wl = x - sb.windows[0]
                        for t4 in range(4):
                            nc.vector.tensor_copy(
                                out=bd[t4 * W:(t4 + 1) * W,
                                       t4 * HP:(t4 + 1) * HP],
                                in_=ad4[t4 * W:(t4 + 1) * W,
                                        wl * HP:(wl + 1) * HP])
                        ti = 0
                        first = True
                        for (rc0, rn) in runs:
                            for g0 in range(0, rn, 4):
                                g = min(4, rn - g0)
                                cols = rc0 + g0 - sb.col0
                                S = sp.tile([128, 4 * W], BF16, tag="S")
                                nc.vector.tensor_tensor(
                                    out=S[:, 0:g * W].rearrange(
                                        "p (t w) -> p t w", w=W),
                                    in0=dstrel_sb[:, rc0 + g0:rc0 + g0 + g]
                                    .unsqueeze(2).broadcast_to([128, g, W]),
                                    in1=iota[:].unsqueeze(1)
                                    .broadcast_to([128, g, W]),
                                    op=OP.is_equal)
                                if "T" not in parts:
                                    continue
                                st_ps = pp.tile([128, 128], BF16, tag="st")
                                nc.tensor.transpose(out=st_ps[0:g * W, :],
                                                    in_=S[:, 0:g * W],
                                                    identity=id_bf[:])
                                st = sp.tile([128, 128], BF16, tag="sts")
                                nc.vector.tensor_copy(out=st[0:g * W, :],
                                                      in_=st_ps[0:g * W, :])
                                if "ad" not in parts:
                                    continue
                                ade_ps = pp.tile([128, 4 * 8], F32, tag="ade")
                                nc.tensor.matmul(out=ade_ps[:, 0:g * HP],
                                                 lhsT=st[0:g * W, :],
                                                 rhs=bd[0:g * W, 0:g * HP],
                                                 start=True, stop=True)
                                u = sp.tile([128, 4 * 8], F32, tag="u")
                                nc.vector.tensor_tensor(
                                    out=u[:, 0:g * H].rearrange(
                                        "p (t h) -> p t h", h=H),
                                    in0=gbuf[:, cols:cols + g, itl:itl + H],
                                    in1=ade_ps[:, 0:g * HP].rearrange(
                                        "p (t h) -> p t h", h=HP)[:, :, 0:H],
                                    op=OP.add)
                                nc.vector.scalar_tensor_tensor(
                                    out=u[:, 0:g * H], in0=u[:, 0:g * H],
                                    scalar=NEG_SLOPE, in1=u[:, 0:g * H],
                                    op0=OP.mult, op1=OP.max)
                                p = sp.tile([128, 4 * 8], BF16, tag="p")
                                nc.scalar.activation(p[:, 0:g * H], u[:, 0:g * H],
                                                     AF.Exp)
                                msg = sp.tile([128, 4 * 85], BF16, tag="msg")
                                nc.vector.tensor_tensor(
                                    out=msg[:, 0:g * itl].rearrange(
                                        "p (t h j) -> p t h j", h=H, j=OO),
                                    in0=gbuf[:, cols:cols + g, 0:itl].rearrange(
                                        "p t (h j) -> p t h j", j=OO),
                                    in1=p[:, 0:g * H].rearrange(
                                        "p (t h) -> p t h", h=H)
                                    .unsqueeze(3).broadcast_to([128, g, H, OO]),
                                    op=OP.mult)
                                if "agg" not in parts:
                                    continue
                                for t in range(g):
                                    nc.tensor.matmul(
                                        out=psum[:, :],
                                        lhsT=S[:, t * W:(t + 1) * W],
                                        rhs=msg[:, t * itl:(t + 1) * itl],
                                        start=first,
                                        stop=(ti == ntile_w - 1))
                                    first = False
                                    ti += 1
                        aw = sp.tile([W, 128], F32, tag="aw")
                        if "agg" in parts:
                            nc.vector.tensor_copy(out=aw[:, :itl], in_=psum[:, :])
                        else:
                            nc.vector.memset(aw[:, :itl], 0.0)
                        nc.sync.dma_start(
                            out=agg_c[l][x * W:(x + 1) * W, :],
                            in_=aw[:, :itl])

            def final_phase():
                H, O, _ = cfg.layers[2]
                OO = O + 1
                for nt in range(cfg.ntiles):
                    r0 = nt * 128
                    ag = sp.tile([128, aggw[2]], F32, tag="ag")
                    nc.sync.dma_start(out=ag[:], in_=agg_c[2][r0:r0 + 128, :])
                    rz = sp.tile([128, 8], F32, tag="rz")
                    nc.vector.tensor_scalar_add(
                        out=rz[:, :H],
                        in0=ag[:].rearrange("p (h j) -> p h j", j=OO)[:, :, 0],
                        scalar1=1e-16)
                    nc.vector.reciprocal(out=rz[:, :H], in_=rz[:, :H])
                    fm = sp.tile([128, O * H], F32, tag="fm")
                    nc.vector.tensor_tensor(
                        out=fm[:].rearrange("p (c h) -> p c h", h=H),
                        in0=ag[:].rearrange("p (h j) -> p h j", j=OO)[:, :, 1:OO]
                        .rearrange("p h c -> p c h"),
                        in1=rz[:, :H].unsqueeze(1).broadcast_to([128, O, H]),
                        op=OP.mult)
                    y = sp.tile([128, O], F32, tag="y")
                    nc.vector.reduce_sum(
                        y[:], fm[:].rearrange("p (c h) -> p c h", h=H), AX.X)
                    nc.vector.tensor_scalar_mul(out=y[:], in0=y[:], scalar1=1.0 / H)
                    nc.vector.tensor_tensor(out=y[:], in0=y[:], in1=b_sb[2][:],
                                            op=OP.add)
                    elu_inplace(y[:], O)
                    mx = sp.tile([128, 1], F32, tag="mx")
                    nc.vector.reduce_sum(mx[:], y[:], AX.X, op=OP.max)
                    nmx = sp.tile([128, 1], F32, tag="nmx")
                    nc.vector.tensor_scalar_mul(out=nmx[:], in0=mx[:], scalar1=-1.0)
                    ey = sp.tile([128, O], F32, tag="ey")
                    nc.scalar.activation(ey[:], y[:], AF.Exp, bias=nmx[:])
                    se = sp.tile([128, 1], F32, tag="se")
                    nc.vector.reduce_sum(se[:], ey[:], AX.X)
                    ls = sp.tile([128, 1], F32, tag="ls")
                    nc.scalar.activation(ls[:], se[:], AF.Ln)
                    nc.vector.tensor_tensor(out=ls[:], in0=ls[:], in1=mx[:],
                                            op=OP.add)
                    yo = sp.tile([128, O], F32, tag="yo")
                    nc.vector.tensor_tensor(
                        out=yo[:], in0=y[:],
                        in1=ls[:].broadcast_to([128, O]), op=OP.subtract)
                    nc.sync.dma_start(out=out[r0:r0 + 128, :], in_=yo[:])

            stop = getattr(cfg, "stop_after", "")
            done = False
            for l in range(3):
                node_phase(l)
                if stop == f"node{l}":
                    done = True; break
                nc.gpsimd.collective_compute(
                    "AllGather", OP.bypass,
                    replica_groups=[list(range(cfg.ncores))],
                    ins=[tbl_c[l][:, :]], outs=[tbl_g[l][:, :]])
                if stop == f"ag{l}":
                    done = True; break
                edge_phase(l)
                if stop == f"edge{l}":
                    done = True; break
            if not done:
                final_phase()

    nc.compile()
    return nc


# ---------------------------------------------------------------------------

def _pad_rows(a, rows):
    o = np.zeros((rows,) + a.shape[1:], np.float32)
    o[:a.shape[0]] = a
    return o


def make_in_maps(cfg, x, Ws, As, Bs):
    W1, W2, W3 = Ws
    ident = np.eye(128, dtype=np.float32)
    iota = np.tile(np.arange(cfg.win, dtype=np.float32), (128, 1))
    xT = np.ascontiguousarray(np.asarray(x, np.float32).T)
    in_maps = []
    for c in range(cfg.ncores):
        m = {}
        xp = np.zeros((cfg.f_pad, cfg.npc), np.float32)
        xp[:cfg.f_in, :cfg.n_real] = xT[:, c * cfg.n_real:(c + 1) * cfg.n_real]
        m["xT"] = xp.astype(BF)
        m["w1"] = _pad_rows(W1, cfg.f_pad).astype(BF)
        m["w2"] = np.asarray(W2, np.float32)
        m["w3"] = np.asarray(W3, np.float32)
        for l, (asv, adv) in enumerate(As):
            m[f"a{l}s"] = np.tile(asv.reshape(1, -1), (128, 1)).astype(np.float32)
            m[f"a{l}d"] = np.tile(adv.reshape(1, -1), (128, 1)).astype(np.float32)
        for i, b in enumerate(Bs):
            m[f"b{i + 1}"] = np.tile(np.asarray(b, np.float32).reshape(1, -1),
                                     (128, 1))
        m["ident_bf"] = ident.astype(BF)
        m["ident_f"] = ident
        m["iota_bf"] = iota.astype(BF)
        m["gidx"] = cfg.gidx[c]
        m["dstrel"] = cfg.dstrel[c].astype(BF)
        in_maps.append(m)
    return in_maps


_CACHE = {}


def run_gat(cfg, nc, x, Ws, As, Bs, **kw):
    in_maps = make_in_maps(cfg, x, Ws, As, Bs)
    res = bass_utils.run_bass_kernel_spmd(nc, in_maps,
                                          core_ids=list(range(cfg.ncores)), **kw)
    outs = [np.asarray(res.results[c]["out"][:cfg.n_real], np.float32)
            for c in range(cfg.ncores)]
    return np.concatenate(outs, axis=0), res


def kernel(x, edge_index, W1, a1s, a1d, b1, W2, a2s, a2d, b2, W3, a3s, a3d, b3):
    x = np.asarray(x, np.float32)
    if "full" not in _CACHE:
        cfg = make_cfg(n=x.shape[0], f_in=x.shape[1])
        prep_edges(cfg, np.asarray(edge_index))
        nc = build(cfg)
        _CACHE["full"] = (cfg, nc)
    cfg, nc = _CACHE["full"]
    As = [(np.asarray(a, np.float32).reshape(-1), np.asarray(d, np.float32).reshape(-1))
          for a, d in ((a1s, a1d), (a2s, a2d), (a3s, a3d))]
    out, _ = run_gat(cfg, nc, x, (np.asarray(W1, np.float32),
                                  np.asarray(W2, np.float32),
                                  np.asarray(W3, np.float32)),
                     As, (b1, b2, b3))
    return out
